# revision 1
# baseline (speedup 1.0000x reference)
"""Trainium2 Bass kernel for nn_ExpertGroup (MoE routing with shared MLP path).

Math (per token t, reference semantics):
    h   = silu(x @ W_up.T)                        [T, H]
    a   = h @ W_adapt.T                           [T, A]
    a_e = a @ W_exp_adapters[e].T  (per expert)   [T, E, A]
    sel = a_{last active expert}                  [T, A]
    an  = LayerNorm(sel) * gamma[e] + beta[e]     [T, A]
    h2  = an @ W_expert_proj.T                    [T, H]
    out = h @ W_out.T + 0.1 * mask * (h2 @ W_out.T)

Key folds:
  * out = (h + h2') @ W_out.T with h2' = 0.1*mask*h2 -- saves a full
    [T,H]@[H,D] matmul.  The 0.1*mask scaling is folded into the gathered
    gamma/beta (an all-zero routing row already zeroes the expert path).
  * All matmuls run as float32r (TF32-like, 1 cycle/row on TRN2 when the
    moving dim >= 256) -- ~4x faster than plain fp32, rel err ~2e-4.

Distribution: pure data parallel over tokens, 8 cores x 1024 tokens.
Activations are kept feature-major [feature(partition), token(free)] so the
whole chain contracts over the partition dim without transposes; only the
small [128,256] LayerNorm/select middle runs token-major (entered for free
via the adapter matmul, exited via PE transposes).

DMA queueing: nc.sync (HW DGE, one in-order queue) carries the
latency-critical weight strip streams; nc.gpsimd (SW DGE, separate queue)
carries bulk prefetches (x, adapter weights, W_out strips) so they never
head-of-line block the strips.
"""

import sys

sys.path.insert(0, "/opt/trn_rl_repo")

from contextlib import ExitStack

import numpy as np

import concourse.bacc as bacc
import concourse.tile as tile
from concourse import mybir
from concourse.masks import make_identity

# Problem shapes (hardcoded per contest contract)
B, S, D = 4, 2048, 1024
H = 4 * D  # 4096
A = H // 16  # 256
E = 8
NCORES = 8
T = B * S  # 8192
TL = T // NCORES  # 1024 tokens per core
LN_EPS = 1e-5

P = 128
KD = D // P  # 8
KH = H // P  # 32
KA = A // P  # 2
TT = TL // P  # 8 token tiles
NTC = TL // 512  # 2 moving-dim chunks

F32 = mybir.dt.float32
F32R = mybir.dt.float32r


def _build():
    nc = bacc.Bacc("TRN2", target_bir_lowering=False, debug=False)
    ACTF = mybir.ActivationFunctionType
    ALU = mybir.AluOpType

    x_d = nc.dram_tensor("x_fm", [D, TL], F32, kind="ExternalInput")
    ew_d = nc.dram_tensor("ew", [TL, E], F32, kind="ExternalInput")
    wup_d = nc.dram_tensor("wup4", [KH, P, KD, P], F32, kind="ExternalInput")
    wad_d = nc.dram_tensor("wadapt_t", [H, A], F32, kind="ExternalInput")
    wex_d = nc.dram_tensor("wexp4", [P, KA, E, A], F32, kind="ExternalInput")
    gam_d = nc.dram_tensor("gamma", [E, A], F32, kind="ExternalInput")
    bet_d = nc.dram_tensor("beta", [E, A], F32, kind="ExternalInput")
    wep_d = nc.dram_tensor("wep4", [P, KA, H], F32, kind="ExternalInput")
    wout_d = nc.dram_tensor("wout4", [KD, P, KH, P], F32, kind="ExternalInput")
    out_d = nc.dram_tensor("out_fm", [D, TL], F32, kind="ExternalOutput")

    with tile.TileContext(nc) as tc, ExitStack() as top:
        pers = top.enter_context(tc.tile_pool(name="pers", bufs=1))
        h = pers.tile([P, KH, TL], F32R, name="h")
        S_oh = pers.tile([P, TT, E], F32, name="S_oh")
        nt = pers.tile([P, TT], F32, name="nt")
        eps_t = pers.tile([P, 1], F32, name="eps_t")
        nc.vector.memset(eps_t[:, :], LN_EPS)

        # right-side stack: expert adapters (live phases 1-3) + wadapt strips,
        # fresh regions so their DMAs run during phase 1 with no slot WAR
        wexwin = ExitStack()
        wex_p = wexwin.enter_context(
            tc.tile_pool(name="wexp", bufs=1, side="right")
        )
        wexp = wex_p.tile([P, KA, E, A], F32R, name="wexp")
        wadwin = ExitStack()
        wad_p = wadwin.enter_context(
            tc.tile_pool(name="wad", bufs=3, side="right")
        )

        # ---- phase 1: h = silu(x @ W_up.T), feature-major ----
        with ExitStack() as p1:
            xp = p1.enter_context(tc.tile_pool(name="xp", bufs=1))
            wup_p = p1.enter_context(tc.tile_pool(name="wup", bufs=3))
            sg_p = p1.enter_context(tc.tile_pool(name="sg", bufs=3))
            pre_p = p1.enter_context(tc.tile_pool(name="pre", bufs=1))
            ps1 = p1.enter_context(tc.tile_pool(name="ps1", bufs=4, space="PSUM"))

            wu_tiles = {}

            def load_wu(hb):
                t = wup_p.tile([P, KD, P], F32R, tag="wu", name=f"wu{hb}")
                nc.sync.dma_start(out=t[:, :, :], in_=wup_d.ap()[hb].bitcast(F32R))
                wu_tiles[hb] = t

            # first weight strip ahead of everything on the sync queue
            load_wu(0)
            # bulk x prefetch on the gpsimd queue, first-needed halves first
            x = xp.tile([P, KD, TL], F32R, name="x")
            xr = x_d.ap().rearrange("(kb p) t -> p kb t", p=P).bitcast(F32R)
            for tcx in range(NTC):
                for kb in range(KD):
                    sl = slice(tcx * 512, (tcx + 1) * 512)
                    nc.gpsimd.dma_start(out=x[:, kb, sl], in_=xr[:, kb, sl])
            load_wu(1)
            load_wu(2)
            # expert adapters ride q0 behind x, landing well before phase 3
            nc.gpsimd.dma_start(out=wexp[:, :, :, :], in_=wex_d.ap().bitcast(F32R))

            # routing one-hot (last active expert wins); small, scheduled
            # into phase-1's shadow
            ewt = pre_p.tile([P, TT, E], F32, name="ewt")
            nc.sync.dma_start(
                out=ewt[:, :, :], in_=ew_d.ap().rearrange("(tt p) e -> p tt e", p=P)
            )
            act_t = pre_p.tile([P, TT, E], F32, name="act_t")
            nc.vector.tensor_scalar(
                out=act_t[:, :, :], in0=ewt[:, :, :], scalar1=0.0, scalar2=None,
                op0=ALU.is_gt,
            )
            nc.vector.memset(nt[:, :], 1.0)
            for e in range(E - 1, -1, -1):
                # S[:,e] = active[:,e] * not_yet_taken; not_yet_taken -= S[:,e]
                nc.vector.tensor_mul(S_oh[:, :, e], act_t[:, :, e], nt[:, :])
                if e:
                    nc.vector.tensor_sub(nt[:, :], nt[:, :], S_oh[:, :, e])

            for hb in range(KH):
                wu = wu_tiles[hb]
                pss = [
                    ps1.tile([P, 512], F32, tag="ps", name=f"ps1_{hb}_{i}")
                    for i in range(NTC)
                ]
                for tcx in range(NTC):
                    for kb in range(KD):
                        nc.tensor.matmul(
                            pss[tcx][:, :],
                            wu[:, kb, :],
                            x[:, kb, tcx * 512 : (tcx + 1) * 512],
                            start=(kb == 0),
                            stop=(kb == KD - 1),
                        )
                if hb + 3 < KH:
                    load_wu(hb + 3)
                for tcx in range(NTC):
                    sg = sg_p.tile([P, 512], F32, tag="sg")
                    nc.scalar.activation(sg[:, :], pss[tcx][:, :], ACTF.Sigmoid)
                    nc.vector.tensor_mul(
                        h[:, hb, tcx * 512 : (tcx + 1) * 512], pss[tcx][:, :], sg[:, :]
                    )

        # ---- window W: W_expert_proj stays resident through phase 4 ----
        win_w = top.enter_context(tc.tile_pool(name="winw", bufs=1))
        wep = win_w.tile([P, KA, H], F32R, name="wep")
        an_fm = win_w.tile([P, KA, TL], F32R, name="an_fm")

        # ---- window A: adapter activations + expert adapter weights ----
        with ExitStack() as win_a:
            mid = win_a.enter_context(tc.tile_pool(name="mid", bufs=1))
            a_fm = mid.tile([P, KA, TL], F32R, name="a_fm")
            ident = mid.tile([P, P], F32, name="ident")
            gb_raw = mid.tile([E, 2 * A], F32, name="gb_raw")
            gb01 = mid.tile([E, 2 * A], F32R, name="gb01")
            make_identity(nc, ident[:, :])
            nc.sync.dma_start(out=gb_raw[:, 0:A], in_=gam_d[:, :])
            nc.sync.dma_start(out=gb_raw[:, A : 2 * A], in_=bet_d[:, :])
            # fold the 0.1 expert-path scale into gathered gamma/beta
            nc.scalar.activation(gb01[:, :], gb_raw[:, :], ACTF.Copy, scale=0.1)

            # ---- phase 2: a = h @ W_adapt.T (stream W_adapt K-strips) ----
            with ExitStack() as p2:
                ps2 = p2.enter_context(tc.tile_pool(name="ps2", bufs=4, space="PSUM"))
                wad_r = wad_d.ap().rearrange("(kb p) o -> p kb o", p=P).bitcast(F32R)
                pa = [
                    ps2.tile([P, 512], F32, tag="pa", name=f"pa_{i}")
                    for i in range(KA * NTC)
                ]
                strip_lens = [1, 1, 2] + [3] * 9 + [1]
                strip_starts = [sum(strip_lens[:i]) for i in range(len(strip_lens))]
                last_strip_dma = None
                for ks, (k0, klen) in enumerate(zip(strip_starts, strip_lens)):
                    wt = wad_p.tile([P, 3, A], F32R, tag="wad", name=f"wad{ks}")
                    last_strip_dma = nc.sync.dma_start(
                        out=wt[:, 0:klen, :], in_=wad_r[:, k0 : k0 + klen, :]
                    )
                    for kx in range(klen):
                        kb = k0 + kx
                        for ob in range(KA):
                            for tcx in range(NTC):
                                nc.tensor.matmul(
                                    pa[ob * NTC + tcx][:, :],
                                    wt[:, kx, ob * P : (ob + 1) * P],
                                    h[:, kb, tcx * 512 : (tcx + 1) * 512],
                                    start=(kb == 0),
                                    stop=(kb == KH - 1),
                                )
                wep_dma = nc.sync.dma_start(
                    out=wep[:, :, :], in_=wep_d.ap().bitcast(F32R)
                )
                # keep the bulk W_expert_proj load behind the latency-critical
                # strip stream on the sync queue (the scheduler would hoist it)
                tile.add_dep_helper(
                    wep_dma.ins, last_strip_dma.ins, sync=False,
                    reason="wep bulk load after wadapt strips",
                )
                wadwin.close()
                for ob in range(KA):
                    for tcx in range(NTC):
                        nc.scalar.activation(
                            a_fm[:, ob, tcx * 512 : (tcx + 1) * 512],
                            pa[ob * NTC + tcx][:, :],
                            ACTF.Copy,
                        )

            # ---- phase 3: adapters + select + LayerNorm (token-major) ----
            with ExitStack() as p3:
                aall_p = p3.enter_context(
                    tc.tile_pool(name="aall", bufs=4, space="PSUM")
                )
                sm_p = p3.enter_context(tc.tile_pool(name="sm", bufs=3, space="PSUM"))
                asel_p = p3.enter_context(tc.tile_pool(name="asel", bufs=2))
                antm_p = p3.enter_context(tc.tile_pool(name="antm", bufs=2))
                st_p = p3.enter_context(tc.tile_pool(name="st", bufs=4))
                stat_p = p3.enter_context(tc.tile_pool(name="stat", bufs=4))

                s_ts = {}

                def prep_st(tt):
                    # transpose the one-hot [128,E] -> [E,128] for the gather
                    pst = sm_p.tile([E, P], F32, tag="sm", name=f"pst{tt}")
                    nc.tensor.transpose(pst[:, :], S_oh[:, tt, :], ident[:, :])
                    s_t = st_p.tile([E, P], F32R, tag="st", name=f"st{tt}")
                    nc.scalar.activation(s_t[:, :], pst[:, :], ACTF.Copy)
                    s_ts[tt] = s_t

                # fill the phase-2 -> phase-3 PE dependency bubble
                for tt in range(2):
                    prep_st(tt)

                for tt in range(TT):
                    t0 = tt * P
                    if tt + 2 < TT:
                        prep_st(tt + 2)
                    # gather 0.1*gamma|0.1*beta rows for each token
                    pg = sm_p.tile([P, 2 * A], F32, tag="sm", name=f"pg{tt}")
                    nc.tensor.matmul(
                        pg[:, :], s_ts[tt][:, :], gb01[:, :], start=True, stop=True
                    )
                    # all-experts adapter matmuls (expert pairs -> N=512);
                    # select with per-token one-hot scalars
                    asel = asel_p.tile([P, A], F32, tag="asel")
                    for ep in range(E // 2):
                        pae = aall_p.tile([P, 2 * A], F32, tag="aall")
                        for kb in range(KA):
                            nc.tensor.matmul(
                                pae[:, :],
                                a_fm[:, kb, t0 : t0 + P],
                                wexp[:, kb, 2 * ep : 2 * ep + 2, :],
                                start=(kb == 0),
                                stop=(kb == KA - 1),
                            )
                        for half in range(2):
                            e = 2 * ep + half
                            pae_h = pae[:, half * A : (half + 1) * A]
                            if e == 0:
                                nc.vector.tensor_scalar(
                                    out=asel[:, :], in0=pae_h,
                                    scalar1=S_oh[:, tt, 0:1], scalar2=None,
                                    op0=ALU.mult,
                                )
                            else:
                                nc.vector.scalar_tensor_tensor(
                                    out=asel[:, :], in0=pae_h,
                                    scalar=S_oh[:, tt, e : e + 1], in1=asel[:, :],
                                    op0=ALU.mult, op1=ALU.add,
                                )
                    # LayerNorm stats
                    st6 = stat_p.tile([P, 6], F32, tag="st6")
                    nc.vector.bn_stats(out=st6[:, :], in_=asel[:, :])
                    mv = stat_p.tile([P, 2], F32, tag="mv")
                    nc.vector.bn_aggr(out=mv[:, :], in_=st6[:, :])
                    sq = stat_p.tile([P, 1], F32, tag="sq")
                    nc.scalar.activation(
                        sq[:, :], mv[:, 1:2], ACTF.Sqrt, bias=eps_t[:, :]
                    )
                    rstd = stat_p.tile([P, 1], F32, tag="rstd")
                    nc.vector.reciprocal(rstd[:, :], sq[:, :])
                    # an = ((asel - mu) * g01) * rstd + be01, two fused ops
                    antm = antm_p.tile([P, A], F32, tag="antm")
                    nc.vector.scalar_tensor_tensor(
                        out=antm[:, :], in0=asel[:, :], scalar=mv[:, 0:1],
                        in1=pg[:, 0:A], op0=ALU.subtract, op1=ALU.mult,
                    )
                    nc.vector.scalar_tensor_tensor(
                        out=antm[:, :], in0=antm[:, :], scalar=rstd[:, :],
                        in1=pg[:, A : 2 * A], op0=ALU.mult, op1=ALU.add,
                    )
                    # back to feature-major via PE transpose
                    for ob in range(KA):
                        ptr = sm_p.tile([P, P], F32, tag="sm", name=f"ptr{tt}_{ob}")
                        nc.tensor.transpose(
                            ptr[:, :], antm[:, ob * P : (ob + 1) * P], ident[:, :]
                        )
                        nc.scalar.activation(
                            an_fm[:, ob, t0 : t0 + P], ptr[:, :], ACTF.Copy
                        )

        wexwin.close()

        # ---- window C: W_expert_proj/W_out strips + output staging ----
        with ExitStack() as win_c:
            wout_p = win_c.enter_context(tc.tile_pool(name="wout", bufs=2))
            ob_p = win_c.enter_context(tc.tile_pool(name="outsb", bufs=3))

            # ---- phase 4: h += an @ W_expert_proj.T ----
            # tcx-major: the tcx=0 pass only needs token tiles 0-3, so the PE
            # runs it while the vector engine finishes LayerNorm on tiles 4-7.
            with ExitStack() as p4:
                ps4 = p4.enter_context(tc.tile_pool(name="ps4", bufs=4, space="PSUM"))
                for tcx in range(NTC):
                    for hb in range(KH):
                        ps = ps4.tile([P, 512], F32, tag="ps")
                        for kb in range(KA):
                            nc.tensor.matmul(
                                ps[:, :],
                                wep[:, kb, hb * P : (hb + 1) * P],
                                an_fm[:, kb, tcx * 512 : (tcx + 1) * 512],
                                start=(kb == 0),
                                stop=(kb == KA - 1),
                            )
                        nc.vector.tensor_add(
                            h[:, hb, tcx * 512 : (tcx + 1) * 512],
                            h[:, hb, tcx * 512 : (tcx + 1) * 512],
                            ps[:, :],
                        )

            # ---- phase 5: out = h @ W_out.T ----
            with ExitStack() as p5:
                ps5 = p5.enter_context(tc.tile_pool(name="ps5", bufs=4, space="PSUM"))
                out_r = out_d.ap().rearrange("(db p) t -> p db t", p=P)

                wo_tiles = {}

                def load_wo(db):
                    t = wout_p.tile([P, KH, P], F32R, tag="wo", name=f"wo{db}")
                    nc.gpsimd.dma_start(
                        out=t[:, :, :], in_=wout_d.ap()[db].bitcast(F32R)
                    )
                    wo_tiles[db] = t

                load_wo(0)
                load_wo(1)
                for db in range(KD):
                    wo = wo_tiles[db]
                    pss = [
                        ps5.tile([P, 512], F32, tag="ps", name=f"ps5_{db}_{i}")
                        for i in range(NTC)
                    ]
                    for kb in range(KH):
                        for tcx in range(NTC):
                            nc.tensor.matmul(
                                pss[tcx][:, :],
                                wo[:, kb, :],
                                h[:, kb, tcx * 512 : (tcx + 1) * 512],
                                start=(kb == 0),
                                stop=(kb == KH - 1),
                            )
                    if db + 2 < KD:
                        load_wo(db + 2)
                    for tcx in range(NTC):
                        sl = slice(tcx * 512, (tcx + 1) * 512)
                        osb = ob_p.tile([P, 512], F32, tag="osb")
                        nc.scalar.activation(osb[:, :], pss[tcx][:, :], ACTF.Copy)
                        nc.sync.dma_start(out=out_r[:, db, sl], in_=osb[:, :])

    nc.compile()
    return nc


_NC = None


def _get_nc():
    global _NC
    if _NC is None:
        _NC = _build()
    return _NC


def _prep_inputs(inputs):
    """Host-side sharding + layout prep (pure data movement, no math)."""
    f = np.float32
    x = np.asarray(inputs["x"], dtype=f).reshape(T, D)
    ew = np.asarray(inputs["expert_weights"], dtype=f).reshape(T, E)
    wup_t = np.asarray(inputs["W_up"], dtype=f).T  # [D, H]
    wad_t = np.asarray(inputs["W_adapt"], dtype=f).T  # [H, A]
    wexp_t = np.asarray(inputs["W_exp_adapters"], dtype=f).transpose(0, 2, 1)  # e,d,o
    gam = np.ascontiguousarray(np.asarray(inputs["ln_gamma"], dtype=f))
    bet = np.ascontiguousarray(np.asarray(inputs["ln_beta"], dtype=f))
    wep_t = np.asarray(inputs["W_expert_proj"], dtype=f).T  # [A, H]
    wout_t = np.asarray(inputs["W_out"], dtype=f).T  # [H, D]

    wup4 = np.ascontiguousarray(wup_t.reshape(KD, P, KH, P).transpose(2, 1, 0, 3))
    wad_c = np.ascontiguousarray(wad_t)
    wexp4 = np.ascontiguousarray(wexp_t.reshape(E, KA, P, A).transpose(2, 1, 0, 3))
    wep4 = np.ascontiguousarray(wep_t.reshape(KA, P, H).transpose(1, 0, 2))
    wout4 = np.ascontiguousarray(wout_t.reshape(KH, P, KD, P).transpose(2, 1, 0, 3))

    shared = {
        "wup4": wup4,
        "wadapt_t": wad_c,
        "wexp4": wexp4,
        "gamma": gam,
        "beta": bet,
        "wep4": wep4,
        "wout4": wout4,
    }
    in_maps = []
    for c in range(NCORES):
        sl = slice(c * TL, (c + 1) * TL)
        m = dict(shared)
        m["x_fm"] = np.ascontiguousarray(x[sl].T)  # [D, TL]
        m["ew"] = np.ascontiguousarray(ew[sl])  # [TL, E]
        in_maps.append(m)
    return in_maps


def _gather_output(results):
    outs = [np.asarray(r["out_fm"]).T for r in results]  # each [TL, D]
    return np.ascontiguousarray(np.concatenate(outs, axis=0).reshape(B, S, D))


def _install_trace_shims():
    """Wire up the NTFF profiling hook that this deployment's antenv lacks,
    and stub the artifact-bucket upload (no object store in container)."""
    import types

    import antenv
    from concourse import bass_utils

    try:
        from antenv.axon_hooks import get_axon_ntff_profile_hook  # noqa: F401
    except ImportError:
        sys.path.insert(0, "/root/.axon_site")
        from trn_agent_boot.trn_boot import _ntff_profile_via_ctypes

        hook = _ntff_profile_via_ctypes("/opt/axon/libaxon_pjrt.so")
        mod = types.ModuleType("antenv.axon_hooks")
        mod.get_axon_ntff_profile_hook = lambda: hook
        mod.set_axon_ntff_profile_hook = lambda h: None
        sys.modules["antenv.axon_hooks"] = mod
        antenv.axon_hooks = mod

    bass_utils.upload_artifacts = lambda tmpdir: str(tmpdir)


def run(inputs, trace=False, trace_cores=None):
    """Returns (output, BassKernelResults)."""
    from concourse import bass_utils

    if trace:
        _install_trace_shims()
    nc = _get_nc()
    in_maps = _prep_inputs(inputs)
    res = bass_utils.run_bass_kernel_spmd(
        nc,
        in_maps,
        core_ids=list(range(NCORES)),
        trace=trace,
        trace_cores=trace_cores,
    )
    return _gather_output(res.results), res


def kernel(**inputs) -> np.ndarray:
    out, _ = run(inputs)
    return out

